# revision 1
# baseline (speedup 1.0000x reference)
"""GravityAE GNN message-passing kernel for 8 TRN2 NeuronCores (Bass/Tile).

Algorithm (see reference GCN autoencoder):
  scale_k = gamma_k / sqrt(var_k + eps); shift_k = beta_k + (b_k - mean_k)*scale_k
  W1p = W1 * scale1; W2p = W2 * scale2
  dinv[n] = 1/sqrt(in_degree incl self loop)
  xs1 = dinv * (x @ W1p)                     (node table, gathered by src)
  h   = leaky(dinv[d] * segsum_d(xs1[src]) + shift1)
  hw2 = dinv * (h @ W2p)
  z   = leaky(dinv[d] * segsum_d(hw2[src]) + shift2)
  out[e] = sigmoid(z[dst,-1] - ||z[src,:-1] - z[dst,:-1]||)

Distribution: aggregation is dst-sharded (each core owns a contiguous range
of 128-node windows; edges sorted by dst, bucketed per window, padded to a
uniform C_max chunks of 128 edges). Per chunk the segment-sum is an
indicator-matrix matmul accumulated in PSUM:  S[e,m] = (dst_local[e]==m),
PSUM += S^T @ gathered_rows.  AllGather (x8) rebuilds the full node tables
between stages.  Decode is edge-sharded in original order.
"""
import math
import numpy as np

P = 128
EPS = 1e-5


# --------------------------------------------------------------------------
# host-side preprocessing
# --------------------------------------------------------------------------
def _build_host_tables(x, edge_index, n_cores):
    N = x.shape[0]
    E = edge_index.shape[1]
    NW = ((N + P - 1) // P + n_cores - 1) // n_cores * n_cores  # windows, mult of n_cores
    NP = NW * P
    src = edge_index[0].astype(np.int64)
    dst = edge_index[1].astype(np.int64)
    s_all = np.concatenate([src, np.arange(N)])
    d_all = np.concatenate([dst, np.arange(N)])
    deg = np.bincount(d_all, minlength=NP).astype(np.float64)
    dinv = np.zeros(NP, np.float32)
    nz = deg > 0
    dinv[nz] = (1.0 / np.sqrt(deg[nz])).astype(np.float32)

    order = np.argsort(d_all, kind="stable")
    s_sorted = d_sorted = None
    s_sorted = s_all[order]
    d_sorted = d_all[order]
    win_of_edge = d_sorted // P
    counts = np.bincount(win_of_edge, minlength=NW)
    C_max = max(1, int(np.ceil(counts.max() / P)))
    CW = C_max * P

    offs = np.full((NW, CW), NP - 1, np.int32)   # pad slots -> last (all-zero) row
    dstf = np.full((NW, CW), -1.0, np.float32)   # pad slots -> never match iota
    starts = np.zeros(NW + 1, np.int64)
    np.cumsum(counts, out=starts[1:])
    for w in range(NW):
        c = counts[w]
        offs[w, :c] = s_sorted[starts[w] : starts[w] + c]
        dstf[w, :c] = (d_sorted[starts[w] : starts[w] + c] - w * P).astype(np.float32)

    # per-core slot tables, SBUF layout [P, NWc*C_max]
    NWc = NW // n_cores
    offs_core = np.empty((n_cores, P, NWc * C_max), np.int32)
    dstf_core = np.empty((n_cores, P, NWc * C_max), np.float32)
    for c in range(n_cores):
        blk_o = offs[c * NWc : (c + 1) * NWc].reshape(NWc, C_max, P)   # [wl, cc, p]
        blk_d = dstf[c * NWc : (c + 1) * NWc].reshape(NWc, C_max, P)
        offs_core[c] = blk_o.transpose(2, 0, 1).reshape(P, NWc * C_max)
        dstf_core[c] = blk_d.transpose(2, 0, 1).reshape(P, NWc * C_max)

    # decode tables: edges in original order, sharded contiguously
    EC = (E + n_cores - 1) // n_cores          # real edges per core (last short)
    DG = (EC + 2047) // 2048                   # groups of 2048 edges
    didx_core = np.zeros((n_cores, P, DG * 32), np.int32)
    for c in range(n_cores):
        e0, e1 = c * EC, min((c + 1) * EC, E)
        n = e1 - e0
        sp = np.zeros(DG * 2048, np.int64)
        dp = np.zeros(DG * 2048, np.int64)
        sp[:n] = src[e0:e1]
        dp[:n] = dst[e0:e1]
        sp3 = sp.reshape(DG, 16, P)            # [g, cc, p]
        dp3 = dp.reshape(DG, 16, P)
        blk = np.concatenate([sp3, dp3], axis=1)      # [g, 32, p]
        didx_core[c] = blk.transpose(2, 0, 1).reshape(P, DG * 32)

    return dict(N=N, E=E, NW=NW, NP=NP, C_max=C_max, NWc=NWc, EC=EC, DG=DG,
                dinv=dinv, offs_core=offs_core, dstf_core=dstf_core,
                didx_core=didx_core)


# --------------------------------------------------------------------------
# bass program
# --------------------------------------------------------------------------
def _build_program(NP, NWc, C_max, F1, F2, DG, n_cores, big_iseq=True):
    import concourse.bass as bass
    import concourse.tile as tile
    from concourse import bacc, mybir

    dt = mybir.dt
    f32 = dt.float32
    i32 = dt.int32
    Nc = NWc * P
    CW = C_max * P
    OB = (DG * 16 + P - 1) // P                 # output transpose blocks
    OUTLEN = OB * P * P

    nc = bacc.Bacc("TRN2", target_bir_lowering=False, debug=False,
                   num_devices=n_cores)
    x_in = nc.declare_dram_parameter("x", [Nc, F1], f32, isOutput=False)
    w1_in = nc.declare_dram_parameter("w1", [F1, F1], f32, isOutput=False)
    w2_in = nc.declare_dram_parameter("w2", [F1, F2], f32, isOutput=False)
    sh1_in = nc.declare_dram_parameter("shift1", [P, F1], f32, isOutput=False)
    sh2_in = nc.declare_dram_parameter("shift2", [P, F2], f32, isOutput=False)
    iota_in = nc.declare_dram_parameter("iota", [P, CW], f32, isOutput=False)
    id_in = nc.declare_dram_parameter("ident", [P, P], f32, isOutput=False)
    dinv_in = nc.declare_dram_parameter("dinv", [Nc, 1], f32, isOutput=False)
    offs_in = nc.declare_dram_parameter("offs", [P, NWc * C_max], i32, isOutput=False)
    dstf_in = nc.declare_dram_parameter("dstf", [P, NWc * C_max], f32, isOutput=False)
    didx_in = nc.declare_dram_parameter("didx", [P, DG * 32], i32, isOutput=False)
    out_dram = nc.declare_dram_parameter("out", [OUTLEN], f32, isOutput=True)

    rg = [list(range(n_cores))]

    with tile.TileContext(nc) as tc:
        with (
            tc.tile_pool(name="const", bufs=1) as cpool,
            tc.tile_pool(name="sbuf", bufs=3) as pool,
            tc.tile_pool(name="psA", bufs=2, space="PSUM") as psA,
            tc.tile_pool(name="dram", bufs=1, space="DRAM") as dpool,
        ):
            # ---- constants ----
            w1_t = cpool.tile([F1, F1], f32)
            w2_t = cpool.tile([F1, F2], f32)
            sh1_t = cpool.tile([P, F1], f32)
            sh2_t = cpool.tile([P, F2], f32)
            iota_t = cpool.tile([P, CW], f32)
            id_t = cpool.tile([P, P], f32)
            nc.sync.dma_start(out=w1_t[:], in_=w1_in[:])
            nc.sync.dma_start(out=w2_t[:], in_=w2_in[:])
            nc.sync.dma_start(out=sh1_t[:], in_=sh1_in[:])
            nc.sync.dma_start(out=sh2_t[:], in_=sh2_in[:])
            nc.sync.dma_start(out=iota_t[:], in_=iota_in[:])
            nc.sync.dma_start(out=id_t[:], in_=id_in[:])

            # ---- collective buffers ----
            ag1_in = dpool.tile([Nc, F1], f32)
            xs1_full = dpool.tile([NP, F1], f32, addr_space="Shared")
            ag2_in = dpool.tile([Nc, F2], f32)
            hw2_full = dpool.tile([NP, F2], f32, addr_space="Shared")
            ag3_in = dpool.tile([Nc, F2], f32)
            z_full = dpool.tile([NP, F2], f32, addr_space="Shared")

            # ---- stage A: xs1 shard = dinv * (x @ W1p) ----
            for w in range(NWc):
                x_t = pool.tile([P, F1], f32, tag="ax")
                dv_t = pool.tile([P, 1], f32, tag="adv")
                nc.sync.dma_start(out=x_t[:], in_=x_in[w * P:(w + 1) * P, :])
                nc.sync.dma_start(out=dv_t[:], in_=dinv_in[w * P:(w + 1) * P, :])
                ps_x = psA.tile([P, P], f32, tag="xp")
                nc.tensor.transpose(ps_x[:], x_t[:], id_t[:])
                xT_t = pool.tile([P, P], f32, tag="axT")
                nc.vector.tensor_copy(xT_t[:], ps_x[:])
                ps_mm = psA.tile([P, F1], f32, tag="mm")
                nc.tensor.matmul(ps_mm[:], xT_t[:], w1_t[:], start=True, stop=True)
                xs_t = pool.tile([P, F1], f32, tag="axs")
                nc.vector.tensor_scalar_mul(xs_t[:], ps_mm[:], dv_t[:, :1])
                nc.sync.dma_start(out=ag1_in[w * P:(w + 1) * P, :], in_=xs_t[:])

            nc.gpsimd.collective_compute(
                "AllGather", mybir.AluOpType.bypass,
                ins=[ag1_in.opt()], outs=[xs1_full.opt()], replica_groups=rg)

            # ---- aggregation layer (shared code for L1 / L2) ----
            def agg_layer(table_full, F, w_t, sh_t, store_cb, tagp):
                for w in range(NWc):
                    of_t = pool.tile([P, C_max], i32, tag=tagp + "of")
                    df_t = pool.tile([P, C_max], f32, tag=tagp + "df")
                    dv_t = pool.tile([P, 1], f32, tag=tagp + "dv")
                    nc.sync.dma_start(out=of_t[:], in_=offs_in[:, w * C_max:(w + 1) * C_max])
                    nc.sync.dma_start(out=df_t[:], in_=dstf_in[:, w * C_max:(w + 1) * C_max])
                    nc.sync.dma_start(out=dv_t[:], in_=dinv_in[w * P:(w + 1) * P, :])
                    msg_t = pool.tile([P, C_max, F], f32, tag=tagp + "msg")
                    for c in range(C_max):
                        nc.gpsimd.indirect_dma_start(
                            out=msg_t[:, c, :], out_offset=None,
                            in_=table_full[:],
                            in_offset=bass.IndirectOffsetOnAxis(ap=of_t[:, c:c + 1], axis=0))
                    S_t = pool.tile([P, C_max, P], f32, tag=tagp + "S")
                    if big_iseq:
                        nc.vector.tensor_tensor(
                            out=S_t[:],
                            in0=df_t[:].rearrange("p (c o) -> p c o", o=1).to_broadcast([P, C_max, P]),
                            in1=iota_t[:].rearrange("p (c m) -> p c m", m=P),
                            op=mybir.AluOpType.is_equal)
                    else:
                        for c in range(C_max):
                            nc.vector.tensor_tensor(
                                out=S_t[:, c, :],
                                in0=df_t[:, c:c + 1].to_broadcast([P, P]),
                                in1=iota_t[:, :P],
                                op=mybir.AluOpType.is_equal)
                    ps_agg = psA.tile([P, F], f32, tag="agg")
                    for c in range(C_max):
                        nc.tensor.matmul(ps_agg[:], S_t[:, c, :], msg_t[:, c, :],
                                         start=(c == 0), stop=(c == C_max - 1))
                    t1 = pool.tile([P, F], f32, tag=tagp + "t1")
                    nc.vector.tensor_scalar_mul(t1[:], ps_agg[:], dv_t[:, :1])
                    t2 = pool.tile([P, F], f32, tag=tagp + "t2")
                    nc.vector.tensor_tensor(out=t2[:], in0=t1[:], in1=sh_t[:],
                                            op=mybir.AluOpType.add)
                    u_t = pool.tile([P, F], f32, tag=tagp + "u")
                    nc.scalar.activation(u_t[:], t2[:],
                                         mybir.ActivationFunctionType.Copy,
                                         scale=0.1)
                    o_t = pool.tile([P, F], f32, tag=tagp + "o")
                    nc.vector.tensor_tensor(out=o_t[:], in0=t2[:], in1=u_t[:],
                                            op=mybir.AluOpType.max)
                    store_cb(w, o_t, dv_t)

            # ---- stage B: h windows + hw2 shard ----
            def store_h(w, h_t, dv_t):
                ps_hx = psA.tile([P, P], f32, tag="xp")
                nc.tensor.transpose(ps_hx[:], h_t[:], id_t[:])
                hT_t = pool.tile([P, P], f32, tag="bhT")
                nc.vector.tensor_copy(hT_t[:], ps_hx[:])
                ps_m2 = psA.tile([P, F2], f32, tag="mm")
                nc.tensor.matmul(ps_m2[:], hT_t[:], w2_t[:], start=True, stop=True)
                hw_t = pool.tile([P, F2], f32, tag="bhw")
                nc.vector.tensor_scalar_mul(hw_t[:], ps_m2[:], dv_t[:, :1])
                nc.sync.dma_start(out=ag2_in[w * P:(w + 1) * P, :], in_=hw_t[:])

            agg_layer(xs1_full, F1, w1_t, sh1_t, store_h, "b")

            nc.gpsimd.collective_compute(
                "AllGather", mybir.AluOpType.bypass,
                ins=[ag2_in.opt()], outs=[hw2_full.opt()], replica_groups=rg)

            # ---- stage C: z shard ----
            def store_z(w, z_t, dv_t):
                nc.sync.dma_start(out=ag3_in[w * P:(w + 1) * P, :], in_=z_t[:])

            agg_layer(hw2_full, F2, w2_t, sh2_t, store_z, "c")

            nc.gpsimd.collective_compute(
                "AllGather", mybir.AluOpType.bypass,
                ins=[ag3_in.opt()], outs=[z_full.opt()], replica_groups=rg)

            # ---- decode ----
            Fp = F2 - 1  # position dims
            stage_ss = cpool.tile([P, OB * P], f32)
            stage_mj = cpool.tile([P, OB * P], f32)
            nc.vector.memset(stage_ss[:], 0.0)
            nc.vector.memset(stage_mj[:], 0.0)
            for g in range(DG):
                di_t = pool.tile([P, 32], i32, tag="ddi")
                nc.sync.dma_start(out=di_t[:], in_=didx_in[:, g * 32:(g + 1) * 32])
                zz_t = pool.tile([P, 32, F2], f32, tag="dzz")
                for c in range(32):
                    nc.gpsimd.indirect_dma_start(
                        out=zz_t[:, c, :], out_offset=None,
                        in_=z_full[:],
                        in_offset=bass.IndirectOffsetOnAxis(ap=di_t[:, c:c + 1], axis=0))
                df_t = pool.tile([P, 16, Fp], f32, tag="ddf")
                nc.vector.tensor_tensor(out=df_t[:], in0=zz_t[:, 0:16, 0:Fp],
                                        in1=zz_t[:, 16:32, 0:Fp],
                                        op=mybir.AluOpType.subtract)
                sq_t = pool.tile([P, 16, Fp], f32, tag="dsq")
                nc.scalar.square(sq_t[:], df_t[:])
                nc.vector.reduce_sum(
                    out=stage_ss[:, g * 16:(g + 1) * 16].rearrange("p (c o) -> p c o", o=1),
                    in_=sq_t[:], axis=mybir.AxisListType.X)
                nc.vector.tensor_copy(stage_mj[:, g * 16:(g + 1) * 16],
                                   zz_t[:, 16:32, Fp])
            # finale: sigmoid(mj - sqrt(ss)), transpose, store
            st_d = cpool.tile([P, OB * P], f32)
            nc.scalar.sqrt(st_d[:], stage_ss[:])
            st_v = cpool.tile([P, OB * P], f32)
            nc.vector.tensor_tensor(out=st_v[:], in0=stage_mj[:], in1=st_d[:],
                                    op=mybir.AluOpType.subtract)
            st_o = cpool.tile([P, OB * P], f32)
            nc.scalar.activation(st_o[:], st_v[:],
                                 mybir.ActivationFunctionType.Sigmoid)
            for b in range(OB):
                ps_t = psA.tile([P, P], f32, tag="xp")
                nc.tensor.transpose(ps_t[:], st_o[:, b * P:(b + 1) * P], id_t[:])
                ob_t = pool.tile([P, P], f32, tag="dob")
                nc.vector.tensor_copy(ob_t[:], ps_t[:])
                nc.sync.dma_start(
                    out=out_dram[b * P * P:(b + 1) * P * P].rearrange("(a b) -> a b", b=P),
                    in_=ob_t[:])
    nc.compile()
    return nc


# --------------------------------------------------------------------------
# public entry
# --------------------------------------------------------------------------
def _prep_inputs(x, edge_index, W1, b1, gamma1, beta1, mean1, var1,
                 W2, b2, gamma2, beta2, mean2, var2, n_cores):
    x = np.asarray(x, np.float32)
    edge_index = np.asarray(edge_index)
    ht = _build_host_tables(x, edge_index, n_cores)
    NP, NWc, C_max, DG = ht["NP"], ht["NWc"], ht["C_max"], ht["DG"]
    F1 = W1.shape[1]
    F2 = W2.shape[1]
    Nc = NWc * P
    CW = C_max * P

    scale1 = np.asarray(gamma1) / np.sqrt(np.asarray(var1) + EPS)
    shift1 = (np.asarray(beta1) + (np.asarray(b1) - np.asarray(mean1)) * scale1).astype(np.float32)
    W1p = (np.asarray(W1) * scale1[None, :]).astype(np.float32)
    scale2 = np.asarray(gamma2) / np.sqrt(np.asarray(var2) + EPS)
    shift2 = (np.asarray(beta2) + (np.asarray(b2) - np.asarray(mean2)) * scale2).astype(np.float32)
    W2p = (np.asarray(W2) * scale2[None, :]).astype(np.float32)

    xp = np.zeros((NP, F1), np.float32)
    xp[: ht["N"]] = x
    iota = np.tile(np.arange(P, dtype=np.float32)[None, :], (1, C_max))  # [1, CW]
    iota = np.broadcast_to(iota, (P, CW)).copy()
    ident = np.eye(P, dtype=np.float32)
    sh1_rep = np.broadcast_to(shift1[None, :], (P, F1)).copy()
    sh2_rep = np.broadcast_to(shift2[None, :], (P, F2)).copy()

    in_maps = []
    for c in range(n_cores):
        in_maps.append({
            "x": np.ascontiguousarray(xp[c * Nc:(c + 1) * Nc]),
            "w1": W1p, "w2": W2p,
            "shift1": sh1_rep, "shift2": sh2_rep,
            "iota": iota, "ident": ident,
            "dinv": np.ascontiguousarray(ht["dinv"][c * Nc:(c + 1) * Nc, None]),
            "offs": ht["offs_core"][c],
            "dstf": ht["dstf_core"][c],
            "didx": ht["didx_core"][c],
        })
    dims = dict(NP=NP, NWc=NWc, C_max=C_max, F1=F1, F2=F2, DG=DG)
    return ht, dims, in_maps


def kernel(x, edge_index, W1, b1, gamma1, beta1, mean1, var1,
           W2, b2, gamma2, beta2, mean2, var2, n_cores=8, _trace=False):
    from concourse.bass_utils import run_bass_kernel_spmd

    ht, dims, in_maps = _prep_inputs(
        x, edge_index, W1, b1, gamma1, beta1, mean1, var1,
        W2, b2, gamma2, beta2, mean2, var2, n_cores)
    nc = _build_program(dims["NP"], dims["NWc"], dims["C_max"],
                        dims["F1"], dims["F2"], dims["DG"], n_cores)
    try:
        res = run_bass_kernel_spmd(nc, in_maps, list(range(n_cores)), trace=_trace)
    except ModuleNotFoundError:
        res = run_bass_kernel_spmd(nc, in_maps, list(range(n_cores)), trace=False)
    E, EC = ht["E"], ht["EC"]
    out = np.empty(E, np.float32)
    for c in range(n_cores):
        e0, e1 = c * EC, min((c + 1) * EC, E)
        out[e0:e1] = res.results[c]["out"][: e1 - e0]
    kernel._last_results = res
    return out



# revision 8
# speedup vs baseline: 733.0477x; 733.0477x over previous
"""GravityAE GNN message-passing kernel for 8 TRN2 NeuronCores (Bass/Tile).

Algorithm (see reference GCN autoencoder):
  scale_k = gamma_k / sqrt(var_k + eps); shift_k = beta_k + (b_k - mean_k)*scale_k
  W1p = W1 * scale1; W2p = W2 * scale2
  dinv[n] = 1/sqrt(in_degree incl self loop)
  xs1 = dinv * (x @ W1p)                     (node table, gathered by src)
  h   = leaky(dinv[d] * segsum_d(xs1[src]) + shift1)
  hw2 = dinv * (h @ W2p)
  z   = leaky(dinv[d] * segsum_d(hw2[src]) + shift2)
  out[e] = sigmoid(z[dst,-1] - ||z[src,:-1] - z[dst,:-1]||)

Distribution: aggregation is dst-sharded (each core owns a contiguous range
of 128-node windows; edges sorted by dst, bucketed per window, padded to a
uniform C_max chunks of 128 edges). Per chunk the segment-sum is an
indicator-matrix matmul accumulated in PSUM:  S[e,m] = (dst_local[e]==m),
PSUM += S^T @ gathered_rows.  AllGather (x8) rebuilds the full node tables
between stages.  Decode is edge-sharded in original order.
"""
import math
import numpy as np

P = 128
EPS = 1e-5


# --------------------------------------------------------------------------
# host-side preprocessing
# --------------------------------------------------------------------------
def _build_host_tables(x, edge_index, n_cores):
    N = x.shape[0]
    E = edge_index.shape[1]
    NW = ((N + P - 1) // P + n_cores - 1) // n_cores * n_cores  # windows, mult of n_cores
    NP = NW * P
    src = edge_index[0].astype(np.int64)
    dst = edge_index[1].astype(np.int64)
    s_all = np.concatenate([src, np.arange(N)])
    d_all = np.concatenate([dst, np.arange(N)])
    deg = np.bincount(d_all, minlength=NP).astype(np.float64)
    dinv = np.zeros(NP, np.float32)
    nz = deg > 0
    dinv[nz] = (1.0 / np.sqrt(deg[nz])).astype(np.float32)

    order = np.argsort(d_all, kind="stable")
    s_sorted = d_sorted = None
    s_sorted = s_all[order]
    d_sorted = d_all[order]
    win_of_edge = d_sorted // P
    counts = np.bincount(win_of_edge, minlength=NW)
    C_max = max(1, int(np.ceil(counts.max() / P)))
    CW = C_max * P

    offs = np.full((NW, CW), NP - 1, np.int32)   # pad slots -> last (all-zero) row
    dstf = np.full((NW, CW), -1.0, np.float32)   # pad slots -> never match iota
    starts = np.zeros(NW + 1, np.int64)
    np.cumsum(counts, out=starts[1:])
    for w in range(NW):
        c = counts[w]
        offs[w, :c] = s_sorted[starts[w] : starts[w] + c]
        dstf[w, :c] = (d_sorted[starts[w] : starts[w] + c] - w * P).astype(np.float32)

    # per-core slot tables, SBUF layout [P, NWc*C_max]
    NWc = NW // n_cores
    offs_core = np.empty((n_cores, P, NWc * C_max), np.int32)
    dstf_core = np.empty((n_cores, P, NWc * C_max), np.float32)
    for c in range(n_cores):
        blk_o = offs[c * NWc : (c + 1) * NWc].reshape(NWc, C_max, P)   # [wl, cc, p]
        blk_d = dstf[c * NWc : (c + 1) * NWc].reshape(NWc, C_max, P)
        offs_core[c] = blk_o.transpose(2, 0, 1).reshape(P, NWc * C_max)
        dstf_core[c] = blk_d.transpose(2, 0, 1).reshape(P, NWc * C_max)

    # decode tables: edges in original order, sharded contiguously
    EC = (E + n_cores - 1) // n_cores          # real edges per core (last short)
    DG = (EC + 2047) // 2048                   # groups of 2048 edges
    didx_core = np.zeros((n_cores, P, DG * 32), np.int32)
    for c in range(n_cores):
        e0, e1 = c * EC, min((c + 1) * EC, E)
        n = e1 - e0
        sp = np.zeros(DG * 2048, np.int64)
        dp = np.zeros(DG * 2048, np.int64)
        sp[:n] = src[e0:e1]
        dp[:n] = dst[e0:e1]
        sp3 = sp.reshape(DG, 16, P)            # [g, cc, p]
        dp3 = dp.reshape(DG, 16, P)
        blk = np.concatenate([sp3, dp3], axis=1)      # [g, 32, p]
        didx_core[c] = blk.transpose(2, 0, 1).reshape(P, DG * 32)

    return dict(N=N, E=E, NW=NW, NP=NP, C_max=C_max, NWc=NWc, EC=EC, DG=DG,
                dinv=dinv, offs_core=offs_core, dstf_core=dstf_core,
                didx_core=didx_core)


# --------------------------------------------------------------------------
# bass program
# --------------------------------------------------------------------------
def _build_program(NP, NWc, C_max, F1, F2, DG, n_cores, big_iseq=True):
    import concourse.bass as bass
    import concourse.tile as tile
    from concourse import bacc, mybir

    dt = mybir.dt
    f32 = dt.float32
    i32 = dt.int32
    Nc = NWc * P
    CW = C_max * P
    OB = (DG * 16 + P - 1) // P                 # output transpose blocks
    OUTLEN = OB * P * P

    nc = bacc.Bacc("TRN2", target_bir_lowering=False, debug=False,
                   num_devices=n_cores)
    x_in = nc.declare_dram_parameter("x", [Nc, F1], f32, isOutput=False)
    w1_in = nc.declare_dram_parameter("w1", [F1, F1], f32, isOutput=False)
    w2_in = nc.declare_dram_parameter("w2", [F1, F2], f32, isOutput=False)
    sh1_in = nc.declare_dram_parameter("shift1", [P, F1], f32, isOutput=False)
    sh2_in = nc.declare_dram_parameter("shift2", [P, F2], f32, isOutput=False)
    iota_in = nc.declare_dram_parameter("iota", [P, CW], f32, isOutput=False)
    id_in = nc.declare_dram_parameter("ident", [P, P], f32, isOutput=False)
    dinv_in = nc.declare_dram_parameter("dinv", [Nc, 1], f32, isOutput=False)
    offs_in = nc.declare_dram_parameter("offs", [P, NWc * C_max], i32, isOutput=False)
    dstf_in = nc.declare_dram_parameter("dstf", [P, NWc * C_max], f32, isOutput=False)
    didx_in = nc.declare_dram_parameter("didx", [P, DG * 32], i32, isOutput=False)
    out_dram = nc.declare_dram_parameter("out", [OUTLEN], f32, isOutput=True)

    rg = [list(range(n_cores))]

    with tile.TileContext(nc) as tc:
        with (
            tc.tile_pool(name="const", bufs=1) as cpool,
            tc.tile_pool(name="sbuf", bufs=3) as pool,
            tc.tile_pool(name="psA", bufs=2, space="PSUM") as psA,
            tc.tile_pool(name="dram", bufs=1, space="DRAM") as dpool,
        ):
            # ---- constants ----
            w1_t = cpool.tile([F1, F1], f32)
            w2_t = cpool.tile([F1, F2], f32)
            sh1_t = cpool.tile([P, F1], f32)
            sh2_t = cpool.tile([P, F2], f32)
            iota_t = cpool.tile([P, CW], f32)
            id_t = cpool.tile([P, P], f32)
            nc.sync.dma_start(out=w1_t[:], in_=w1_in[:])
            nc.sync.dma_start(out=w2_t[:], in_=w2_in[:])
            nc.sync.dma_start(out=sh1_t[:], in_=sh1_in[:])
            nc.sync.dma_start(out=sh2_t[:], in_=sh2_in[:])
            nc.sync.dma_start(out=iota_t[:], in_=iota_in[:])
            nc.sync.dma_start(out=id_t[:], in_=id_in[:])

            # ---- collective buffers ----
            ag1_in = dpool.tile([Nc, F1], f32)
            xs1_full = dpool.tile([NP, F1], f32, addr_space="Shared")
            ag2_in = dpool.tile([Nc, F2], f32)
            hw2_full = dpool.tile([NP, F2], f32, addr_space="Shared")
            ag3_in = dpool.tile([Nc, F2], f32)
            z_full = dpool.tile([NP, F2], f32, addr_space="Shared")

            # ---- stage A: xs1 shard = dinv * (x @ W1p) ----
            sA = nc.enter_named_scope("stageA", notify=True)
            for w in range(NWc):
                x_t = pool.tile([P, F1], f32, tag="ax")
                dv_t = pool.tile([P, 1], f32, tag="adv")
                nc.sync.dma_start(out=x_t[:], in_=x_in[w * P:(w + 1) * P, :])
                nc.sync.dma_start(out=dv_t[:], in_=dinv_in[w * P:(w + 1) * P, :])
                ps_x = psA.tile([P, P], f32, tag="xp")
                nc.tensor.transpose(ps_x[:], x_t[:], id_t[:])
                xT_t = pool.tile([P, P], f32, tag="axT")
                nc.vector.tensor_copy(xT_t[:], ps_x[:])
                ps_mm = psA.tile([P, F1], f32, tag="mm")
                nc.tensor.matmul(ps_mm[:], xT_t[:], w1_t[:], start=True, stop=True)
                xs_t = pool.tile([P, F1], f32, tag="axs")
                nc.vector.tensor_scalar_mul(xs_t[:], ps_mm[:], dv_t[:, :1])
                nc.sync.dma_start(out=ag1_in[w * P:(w + 1) * P, :], in_=xs_t[:])

            nc.leave_named_scope("stageA", sA[0], notify=True)
            sG = nc.enter_named_scope("AG1", notify=True)
            nc.gpsimd.collective_compute(
                "AllGather", mybir.AluOpType.bypass,
                ins=[ag1_in.opt()], outs=[xs1_full.opt()], replica_groups=rg)
            nc.leave_named_scope("AG1", sG[0], notify=True)

            # ---- aggregation layer (shared code for L1 / L2) ----
            def agg_layer(table_full, F, w_t, sh_t, store_cb, tagp):
                for w in range(NWc):
                    of_t = pool.tile([P, C_max], i32, tag=tagp + "of")
                    df_t = pool.tile([P, C_max], f32, tag=tagp + "df")
                    dv_t = pool.tile([P, 1], f32, tag=tagp + "dv")
                    nc.sync.dma_start(out=of_t[:], in_=offs_in[:, w * C_max:(w + 1) * C_max])
                    nc.sync.dma_start(out=df_t[:], in_=dstf_in[:, w * C_max:(w + 1) * C_max])
                    nc.sync.dma_start(out=dv_t[:], in_=dinv_in[w * P:(w + 1) * P, :])
                    msg_t = pool.tile([P, C_max, F], f32, tag=tagp + "msg")
                    for c in range(C_max):
                        nc.gpsimd.indirect_dma_start(
                            out=msg_t[:, c, :], out_offset=None,
                            in_=table_full[:],
                            in_offset=bass.IndirectOffsetOnAxis(ap=of_t[:, c:c + 1], axis=0))
                    S_t = pool.tile([P, C_max, P], f32, tag=tagp + "S")
                    if big_iseq:
                        nc.vector.tensor_tensor(
                            out=S_t[:],
                            in0=df_t[:].rearrange("p (c o) -> p c o", o=1).to_broadcast([P, C_max, P]),
                            in1=iota_t[:].rearrange("p (c m) -> p c m", m=P),
                            op=mybir.AluOpType.is_equal)
                    else:
                        for c in range(C_max):
                            nc.vector.tensor_tensor(
                                out=S_t[:, c, :],
                                in0=df_t[:, c:c + 1].to_broadcast([P, P]),
                                in1=iota_t[:, :P],
                                op=mybir.AluOpType.is_equal)
                    ps_agg = psA.tile([P, F], f32, tag="agg")
                    for c in range(C_max):
                        nc.tensor.matmul(ps_agg[:], S_t[:, c, :], msg_t[:, c, :],
                                         start=(c == 0), stop=(c == C_max - 1))
                    t1 = pool.tile([P, F], f32, tag=tagp + "t1")
                    nc.vector.tensor_scalar_mul(t1[:], ps_agg[:], dv_t[:, :1])
                    t2 = pool.tile([P, F], f32, tag=tagp + "t2")
                    nc.vector.tensor_tensor(out=t2[:], in0=t1[:], in1=sh_t[:],
                                            op=mybir.AluOpType.add)
                    u_t = pool.tile([P, F], f32, tag=tagp + "u")
                    nc.scalar.activation(u_t[:], t2[:],
                                         mybir.ActivationFunctionType.Copy,
                                         scale=0.1)
                    o_t = pool.tile([P, F], f32, tag=tagp + "o")
                    nc.vector.tensor_tensor(out=o_t[:], in0=t2[:], in1=u_t[:],
                                            op=mybir.AluOpType.max)
                    store_cb(w, o_t, dv_t)

            # ---- stage B: h windows + hw2 shard ----
            def store_h(w, h_t, dv_t):
                ps_hx = psA.tile([P, P], f32, tag="xp")
                nc.tensor.transpose(ps_hx[:], h_t[:], id_t[:])
                hT_t = pool.tile([P, P], f32, tag="bhT")
                nc.vector.tensor_copy(hT_t[:], ps_hx[:])
                ps_m2 = psA.tile([P, F2], f32, tag="mm")
                nc.tensor.matmul(ps_m2[:], hT_t[:], w2_t[:], start=True, stop=True)
                hw_t = pool.tile([P, F2], f32, tag="bhw")
                nc.vector.tensor_scalar_mul(hw_t[:], ps_m2[:], dv_t[:, :1])
                nc.sync.dma_start(out=ag2_in[w * P:(w + 1) * P, :], in_=hw_t[:])

            sB = nc.enter_named_scope("aggB", notify=True)
            agg_layer(xs1_full, F1, w1_t, sh1_t, store_h, "b")
            nc.leave_named_scope("aggB", sB[0], notify=True)

            sG2 = nc.enter_named_scope("AG2", notify=True)
            nc.gpsimd.collective_compute(
                "AllGather", mybir.AluOpType.bypass,
                ins=[ag2_in.opt()], outs=[hw2_full.opt()], replica_groups=rg)
            nc.leave_named_scope("AG2", sG2[0], notify=True)

            # ---- stage C: z shard ----
            def store_z(w, z_t, dv_t):
                nc.sync.dma_start(out=ag3_in[w * P:(w + 1) * P, :], in_=z_t[:])

            sC = nc.enter_named_scope("aggC", notify=True)
            agg_layer(hw2_full, F2, w2_t, sh2_t, store_z, "c")
            nc.leave_named_scope("aggC", sC[0], notify=True)

            sG3 = nc.enter_named_scope("AG3", notify=True)
            nc.gpsimd.collective_compute(
                "AllGather", mybir.AluOpType.bypass,
                ins=[ag3_in.opt()], outs=[z_full.opt()], replica_groups=rg)
            nc.leave_named_scope("AG3", sG3[0], notify=True)

            # ---- decode ----
            sD = nc.enter_named_scope("decode", notify=True)
            Fp = F2 - 1  # position dims
            stage_ss = cpool.tile([P, OB * P], f32)
            stage_mj = cpool.tile([P, OB * P], f32)
            nc.vector.memset(stage_ss[:], 0.0)
            nc.vector.memset(stage_mj[:], 0.0)
            for g in range(DG):
                di_t = pool.tile([P, 32], i32, tag="ddi")
                nc.sync.dma_start(out=di_t[:], in_=didx_in[:, g * 32:(g + 1) * 32])
                zz_t = pool.tile([P, 32, F2], f32, tag="dzz")
                for c in range(32):
                    nc.gpsimd.indirect_dma_start(
                        out=zz_t[:, c, :], out_offset=None,
                        in_=z_full[:],
                        in_offset=bass.IndirectOffsetOnAxis(ap=di_t[:, c:c + 1], axis=0))
                df_t = pool.tile([P, 16, Fp], f32, tag="ddf")
                nc.vector.tensor_tensor(out=df_t[:], in0=zz_t[:, 0:16, 0:Fp],
                                        in1=zz_t[:, 16:32, 0:Fp],
                                        op=mybir.AluOpType.subtract)
                sq_t = pool.tile([P, 16, Fp], f32, tag="dsq")
                nc.scalar.square(sq_t[:], df_t[:])
                nc.vector.reduce_sum(
                    out=stage_ss[:, g * 16:(g + 1) * 16].rearrange("p (c o) -> p c o", o=1),
                    in_=sq_t[:], axis=mybir.AxisListType.X)
                nc.vector.tensor_copy(stage_mj[:, g * 16:(g + 1) * 16],
                                   zz_t[:, 16:32, Fp])
            # finale: sigmoid(mj - sqrt(ss)), transpose, store
            st_d = cpool.tile([P, OB * P], f32)
            nc.scalar.sqrt(st_d[:], stage_ss[:])
            st_v = cpool.tile([P, OB * P], f32)
            nc.vector.tensor_tensor(out=st_v[:], in0=stage_mj[:], in1=st_d[:],
                                    op=mybir.AluOpType.subtract)
            st_o = cpool.tile([P, OB * P], f32)
            nc.scalar.activation(st_o[:], st_v[:],
                                 mybir.ActivationFunctionType.Sigmoid)
            for b in range(OB):
                ps_t = psA.tile([P, P], f32, tag="xp")
                nc.tensor.transpose(ps_t[:], st_o[:, b * P:(b + 1) * P], id_t[:])
                ob_t = pool.tile([P, P], f32, tag="dob")
                nc.vector.tensor_copy(ob_t[:], ps_t[:])
                nc.sync.dma_start(
                    out=out_dram[b * P * P:(b + 1) * P * P].rearrange("(a b) -> a b", b=P),
                    in_=ob_t[:])
            nc.leave_named_scope("decode", sD[0], notify=True)
    nc.compile()
    return nc


# --------------------------------------------------------------------------
# public entry
# --------------------------------------------------------------------------
def _prep_inputs(x, edge_index, W1, b1, gamma1, beta1, mean1, var1,
                 W2, b2, gamma2, beta2, mean2, var2, n_cores):
    x = np.asarray(x, np.float32)
    edge_index = np.asarray(edge_index)
    ht = _build_host_tables(x, edge_index, n_cores)
    NP, NWc, C_max, DG = ht["NP"], ht["NWc"], ht["C_max"], ht["DG"]
    F1 = W1.shape[1]
    F2 = W2.shape[1]
    Nc = NWc * P
    CW = C_max * P

    scale1 = np.asarray(gamma1) / np.sqrt(np.asarray(var1) + EPS)
    shift1 = (np.asarray(beta1) + (np.asarray(b1) - np.asarray(mean1)) * scale1).astype(np.float32)
    W1p = (np.asarray(W1) * scale1[None, :]).astype(np.float32)
    scale2 = np.asarray(gamma2) / np.sqrt(np.asarray(var2) + EPS)
    shift2 = (np.asarray(beta2) + (np.asarray(b2) - np.asarray(mean2)) * scale2).astype(np.float32)
    W2p = (np.asarray(W2) * scale2[None, :]).astype(np.float32)

    xp = np.zeros((NP, F1), np.float32)
    xp[: ht["N"]] = x
    iota = np.tile(np.arange(P, dtype=np.float32)[None, :], (1, C_max))  # [1, CW]
    iota = np.broadcast_to(iota, (P, CW)).copy()
    ident = np.eye(P, dtype=np.float32)
    sh1_rep = np.broadcast_to(shift1[None, :], (P, F1)).copy()
    sh2_rep = np.broadcast_to(shift2[None, :], (P, F2)).copy()

    in_maps = []
    for c in range(n_cores):
        in_maps.append({
            "x": np.ascontiguousarray(xp[c * Nc:(c + 1) * Nc]),
            "w1": W1p, "w2": W2p,
            "shift1": sh1_rep, "shift2": sh2_rep,
            "iota": iota, "ident": ident,
            "dinv": np.ascontiguousarray(ht["dinv"][c * Nc:(c + 1) * Nc, None]),
            "offs": ht["offs_core"][c],
            "dstf": ht["dstf_core"][c],
            "didx": ht["didx_core"][c],
        })
    dims = dict(NP=NP, NWc=NWc, C_max=C_max, F1=F1, F2=F2, DG=DG)
    return ht, dims, in_maps


def kernel(x, edge_index, W1, b1, gamma1, beta1, mean1, var1,
           W2, b2, gamma2, beta2, mean2, var2, n_cores=8, _trace=False):
    from concourse.bass_utils import run_bass_kernel_spmd

    ht, dims, in_maps = _prep_inputs(
        x, edge_index, W1, b1, gamma1, beta1, mean1, var1,
        W2, b2, gamma2, beta2, mean2, var2, n_cores)
    nc = _build_program(dims["NP"], dims["NWc"], dims["C_max"],
                        dims["F1"], dims["F2"], dims["DG"], n_cores)
    try:
        res = run_bass_kernel_spmd(nc, in_maps, list(range(n_cores)), trace=_trace,
                                   trace_cores=list(range(n_cores)) if _trace else None)
    except ModuleNotFoundError:
        res = run_bass_kernel_spmd(nc, in_maps, list(range(n_cores)), trace=False)
    E, EC = ht["E"], ht["EC"]
    out = np.empty(E, np.float32)
    for c in range(n_cores):
        e0, e1 = c * EC, min((c + 1) * EC, E)
        out[e0:e1] = res.results[c]["out"][: e1 - e0]
    kernel._last_results = res
    return out

